# revision 14
# baseline (speedup 1.0000x reference)
"""Trainium2 Bass kernel: isometry-regularization loss (nn_IsometryReg).

Math: for a linear classifier l = xW + b (c=10 classes, n=3072 features),
the per-sample Jacobian of y = 2 r[:9] / (1 - r[9])  (r = sqrt(a*softmax(l)+eps))
w.r.t. x factors as  jac = Jl @ W^T  with Jl [9,10] the Jacobian w.r.t. logits:
    Jl[i,j] = alpha_i d_ij + gamma_i d_j9 - (alpha_i+gamma_i) s_j
    alpha_i = a u s_i / r_i,  gamma_i = a u^2 r_i s_9 / r_9,  u = 1/(1-r_9)
Hence G = jac jac^T = Jl (W^T W) Jl^T — the [B,9,3072] Jacobian is never
materialized.  ||G - f I||_F^2 = ||G||^2 - 2 f tr(G) + 9 f^2 (f >> ||G||, no
cancellation), and arccos(x) = arctan(sqrt(1-x^2)/x) for the x>0 range here.

Sharding: pure data-parallel, 128 samples per core on 8 cores; W, b replicated.
Per-core shard is sent pre-laid-out as x^T chunks (xt[p, j*128+b] =
x[b, j*128+p]) so the k-contraction lands on partitions; this is a layout
choice of the shard, the device still reads every byte of x once.
"""

import numpy as np

import concourse.bass as bass
import concourse.tile as tile
from concourse import mybir
from concourse.bass_utils import run_bass_kernel_spmd

F32 = mybir.dt.float32
AX = mybir.AxisListType
OP = mybir.AluOpType
AF = mybir.ActivationFunctionType

B, N, C = 1024, 3072, 10
M = C - 1                      # 9
NCORES = 8
BC = B // NCORES               # 128 samples per core
KCH = N // 128                 # 24 k-chunks
NUM_STAB = 1e-4
A_CONST = 1.0 - C * NUM_STAB   # 0.999
EPSILON = 0.1

_CACHE = {}

# feature toggles for walrus-codegen bisect
USE_PBCAST = True
USE_TTR = False
NDMA = 8


def _build():
    nc = bass.Bass()

    xt = nc.dram_tensor("xt", [128, N], F32, kind="ExternalInput")
    wc = nc.dram_tensor("wc", [128, KCH * C], F32, kind="ExternalInput")
    bv = nc.dram_tensor("bv", [C, 1], F32, kind="ExternalInput")
    id10 = nc.dram_tensor("id10", [C, C], F32, kind="ExternalInput")
    out = nc.dram_tensor("reg", [BC, 1], F32, kind="ExternalOutput")

    with tile.TileContext(nc) as tc:
        with (
            tc.tile_pool(name="const", bufs=1) as const,
            tc.tile_pool(name="xb", bufs=1) as xb,
            tc.tile_pool(name="work", bufs=1) as work,
            tc.tile_pool(name="psum", bufs=1, space="PSUM") as psum,
        ):
            # ---- loads ----
            wc_sb = const.tile([128, KCH * C], F32)
            nc.sync.dma_start(wc_sb[:], wc[:])
            b_sb = const.tile([C, 1], F32)
            nc.sync.dma_start(b_sb[:], bv[:])
            id_sb = const.tile([C, C], F32)
            nc.sync.dma_start(id_sb[:], id10[:])

            xt_sb = xb.tile([128, N], F32)
            cw = N // NDMA
            for d in range(NDMA):
                nc.sync.dma_start(
                    xt_sb[:, d * cw:(d + 1) * cw], xt[:, d * cw:(d + 1) * cw]
                )

            # ---- K = W^T W  [10,10], then broadcast to [128, 100] ----
            kpsum = psum.tile([C, C], F32)
            for j in range(KCH):
                nc.tensor.matmul(
                    kpsum[:],
                    wc_sb[:, j * C:(j + 1) * C],
                    wc_sb[:, j * C:(j + 1) * C],
                    start=(j == 0),
                    stop=(j == KCH - 1),
                )
            k10_sb = const.tile([C, C], F32)
            nc.scalar.copy(k10_sb[:], kpsum[:])
            k1_sb = const.tile([1, C * C], F32)
            nc.sync.dma_start(k1_sb[:], k10_sb[:])
            kbc = const.tile([128, C * C], F32)
            # PE broadcast: ones[1,128]^T @ k1[1,100].  The warmup matmul
            # absorbs the DVE-memset dependency so the broadcast matmul
            # carries a single sync wait (f32 self-loading LDW struct has
            # one wait slot in walrus codegen).
            ones1 = const.tile([1, 128], F32)
            nc.vector.memset(ones1[:], 1.0)
            warm_ps = psum.tile([128, 1], F32)
            nc.tensor.matmul(warm_ps[:], ones1[:], ones1[:, 0:1],
                             start=True, stop=True)
            kbc_ps = psum.tile([128, C * C], F32)
            nc.tensor.matmul(kbc_ps[:], ones1[:], k1_sb[:],
                             start=True, stop=True)
            nc.scalar.copy(kbc[:], kbc_ps[:])

            # ---- logitsT = W^T x^T [10, 128] ----
            lpsum = psum.tile([C, 128], F32)
            for j in range(KCH):
                nc.tensor.matmul(
                    lpsum[:],
                    wc_sb[:, j * C:(j + 1) * C],
                    xt_sb[:, j * 128:(j + 1) * 128],
                    start=(j == 0),
                    stop=(j == KCH - 1),
                )
            lt_sb = work.tile([C, 128], F32)
            # bias add fused with PSUM->SBUF copy
            nc.vector.tensor_scalar_add(lt_sb[:], lpsum[:], b_sb[:])

            # ---- transpose -> logits [128, 10] ----
            l_psum = psum.tile([128, C], F32)
            nc.tensor.transpose(l_psum[:], lt_sb[:], id_sb[:])

            # ---- softmax (batch on partitions) ----
            negmax = work.tile([BC, 1], F32)
            nc.vector.tensor_reduce(
                negmax[:], l_psum[:], axis=AX.X, op=OP.max, negate=True
            )
            E = work.tile([BC, C], F32)
            SE = work.tile([BC, 1], F32)
            nc.scalar.activation(
                E[:], l_psum[:], AF.Exp, bias=negmax[:], scale=1.0, accum_out=SE[:]
            )
            SEr = work.tile([BC, 1], F32)
            nc.vector.reciprocal(SEr[:], SE[:])
            S = work.tile([BC, C], F32)
            nc.scalar.mul(S[:], E[:], SEr[:])

            # r = sqrt(a*s + eps), with accumulated row-sum for delta
            eps_sb = const.tile([BC, 1], F32)
            nc.vector.memset(eps_sb[:], NUM_STAB)
            R = work.tile([BC, C], F32)
            SR = work.tile([BC, 1], F32)
            nc.scalar.activation(
                R[:], S[:], AF.Sqrt, bias=eps_sb[:], scale=A_CONST, accum_out=SR[:]
            )
            Rinv = work.tile([BC, C], F32)
            nc.vector.reciprocal(Rinv[:], R[:])

            # u = 1/(1 - r9), u^2
            OMR = work.tile([BC, 1], F32)
            nc.vector.tensor_scalar(
                OMR[:], R[:, M:C], -1.0, 1.0, op0=OP.mult, op1=OP.add
            )
            U = work.tile([BC, 1], F32)
            nc.vector.reciprocal(U[:], OMR[:])
            U2 = work.tile([BC, 1], F32)
            nc.vector.tensor_mul(U2[:], U[:], U[:])

            # alpha, gamma, -(alpha+gamma)
            SRi = work.tile([BC, M], F32)
            nc.vector.tensor_mul(SRi[:], S[:, :M], Rinv[:, :M])
            ALPHA = work.tile([BC, M], F32)
            nc.vector.tensor_scalar(
                ALPHA[:], SRi[:], U[:], A_CONST, op0=OP.mult, op1=OP.mult
            )
            SR9 = work.tile([BC, 1], F32)
            nc.vector.tensor_mul(SR9[:], S[:, M:C], Rinv[:, M:C])
            G0 = work.tile([BC, 1], F32)
            nc.vector.tensor_scalar(
                G0[:], SR9[:], U2[:], A_CONST, op0=OP.mult, op1=OP.mult
            )
            GAMMA = work.tile([BC, M], F32)
            nc.vector.tensor_scalar_mul(GAMMA[:], R[:, :M], G0[:])
            TAUN = work.tile([BC, M], F32)
            nc.vector.scalar_tensor_tensor(
                TAUN[:], ALPHA[:], -1.0, GAMMA[:], op0=OP.mult, op1=OP.subtract
            )

            # ---- Jl [128, 90]:  -(tau) x s  + diag(alpha) + gamma e9 ----
            JL = work.tile([BC, M * C], F32)
            nc.vector.tensor_mul(
                JL[:].rearrange("p (i j) -> p i j", i=M),
                TAUN[:, :, None].broadcast_to([BC, M, C]),
                S[:, None, :].broadcast_to([BC, M, C]),
            )
            nc.vector.tensor_add(JL[:, 0:M * C:C + 1], JL[:, 0:M * C:C + 1], ALPHA[:])
            nc.vector.tensor_add(
                JL[:, M:M * C:C], JL[:, M:M * C:C], GAMMA[:]
            )

            # ---- TT = Jl K  (per sample): [128, 90] ----
            TTm = work.tile([BC, M * C * C], F32)
            nc.vector.tensor_mul(
                TTm[:].rearrange("p (i k j) -> p i k j", i=M, k=C),
                JL[:].rearrange("p (i j) -> p i j", i=M)[:, :, None, :]
                .broadcast_to([BC, M, C, C]),
                kbc[:].rearrange("p (k j) -> p k j", k=C)[:, None, :, :]
                .broadcast_to([BC, M, C, C]),
            )
            TT = work.tile([BC, M * C], F32)
            nc.vector.tensor_reduce(
                TT[:], TTm[:].rearrange("p (g j) -> p g j", j=C),
                axis=AX.X, op=OP.add,
            )

            # ---- G = TT Jl^T (per sample): [128, 81] ----
            Gm = work.tile([BC, M * M * C], F32)
            nc.vector.tensor_mul(
                Gm[:].rearrange("p (i l k) -> p i l k", i=M, l=M),
                TT[:].rearrange("p (i k) -> p i k", i=M)[:, :, None, :]
                .broadcast_to([BC, M, M, C]),
                JL[:].rearrange("p (l k) -> p l k", l=M)[:, None, :, :]
                .broadcast_to([BC, M, M, C]),
            )
            G = work.tile([BC, M * M], F32)
            nc.vector.tensor_reduce(
                G[:], Gm[:].rearrange("p (g k) -> p g k", k=C), axis=AX.X, op=OP.add
            )

            # ---- ||G||^2 and tr(G) ----
            scrap = work.tile([BC, M * M], F32)
            SSQ = work.tile([BC, 1], F32)
            if USE_TTR:
                nc.vector.tensor_tensor_reduce(
                    out=scrap[:], in0=G[:], in1=G[:], scale=1.0, scalar=0.0,
                    op0=OP.mult, op1=OP.add, accum_out=SSQ[:],
                )
            else:
                nc.vector.tensor_mul(scrap[:], G[:], G[:])
                nc.vector.tensor_reduce(SSQ[:], scrap[:], axis=AX.X, op=OP.add)
            TRG = work.tile([BC, 1], F32)
            nc.vector.tensor_reduce(
                TRG[:], G[:, 0:M * M:M + 1], axis=AX.X, op=OP.add
            )

            # ---- delta = 2 arccos(SR/sqrt(10)) via arctan ----
            X2 = work.tile([BC, 1], F32)
            nc.scalar.activation(X2[:], SR[:], AF.Square, scale=1.0 / np.sqrt(C))
            OMX2 = work.tile([BC, 1], F32)
            nc.vector.tensor_scalar(
                OMX2[:], X2[:], -1.0, 1.0, op0=OP.mult, op1=OP.add
            )
            SQX = work.tile([BC, 1], F32)
            nc.scalar.activation(SQX[:], OMX2[:], AF.Sqrt)
            XV = work.tile([BC, 1], F32)
            nc.vector.tensor_scalar_mul(XV[:], SR[:], float(1.0 / np.sqrt(C)))
            XR = work.tile([BC, 1], F32)
            nc.vector.reciprocal(XR[:], XV[:])
            QT = work.tile([BC, 1], F32)
            nc.vector.tensor_mul(QT[:], SQX[:], XR[:])
            AC = work.tile([BC, 1], F32)
            nc.scalar.activation(AC[:], QT[:], AF.Arctan)

            # ---- f = 100 * AC^2 * u^2 ; res = SSQ - 2 f trG + 9 f^2 ----
            FA = work.tile([BC, 1], F32)
            nc.vector.tensor_mul(FA[:], AC[:], AC[:])
            F = work.tile([BC, 1], F32)
            nc.vector.tensor_scalar(
                F[:], FA[:], U2[:], 100.0, op0=OP.mult, op1=OP.mult
            )
            FT = work.tile([BC, 1], F32)
            nc.vector.tensor_mul(FT[:], F[:], TRG[:])
            R1 = work.tile([BC, 1], F32)
            nc.vector.scalar_tensor_tensor(
                R1[:], FT[:], -2.0, SSQ[:], op0=OP.mult, op1=OP.add
            )
            FF = work.tile([BC, 1], F32)
            nc.vector.tensor_mul(FF[:], F[:], F[:])
            RES = work.tile([BC, 1], F32)
            nc.vector.scalar_tensor_tensor(
                RES[:], FF[:], 9.0, R1[:], op0=OP.mult, op1=OP.add
            )
            REG = work.tile([BC, 1], F32)
            nc.scalar.activation(
                REG[:], RES[:], AF.Sqrt, scale=1.0 / (float(N) * float(N))
            )
            nc.sync.dma_start(out[:], REG[:])

    return nc


def _split_waits(nc):
    """Walrus codegen on this toolchain encodes at most one sync-wait per
    instruction; hoist extra waits onto same-engine NoOps inserted before."""
    for blk in nc.main_func.blocks:
        newlist = []
        changed = False
        for ins in blk.instructions:
            si = getattr(ins, "sync_info", None)
            ow = getattr(si, "on_wait", None) if si is not None else None
            if ow and len(ow) > 1:
                for idx, w in enumerate(ow[:-1]):
                    nop = mybir.InstNoOp(name=f"{ins.name}-sw{idx}", ins=[], outs=[])
                    nop.engine = ins.engine
                    nop.sync_info = mybir.SyncInfo(on_wait=[w], on_update=[])
                    newlist.append(nop)
                si.on_wait = [ow[-1]]
                changed = True
            newlist.append(ins)
        if changed:
            blk.instructions = newlist
    return nc


def _get_nc():
    if "nc" not in _CACHE:
        _CACHE["nc"] = _split_waits(_build())
    return _CACHE["nc"]


def _shard_inputs(data, W, b):
    """Host-side layout: per-core transposed x chunks + chunked W."""
    x = np.ascontiguousarray(np.asarray(data, np.float32).reshape(B, N))
    W = np.asarray(W, np.float32)
    b = np.asarray(b, np.float32)

    # wc[p, j*10+c] = W[j*128+p, c]
    wc = np.ascontiguousarray(
        W.reshape(KCH, 128, C).transpose(1, 0, 2).reshape(128, KCH * C)
    )
    bv = np.ascontiguousarray(b.reshape(C, 1))
    id10 = np.eye(C, dtype=np.float32)

    in_maps = []
    for i in range(NCORES):
        sh = x[i * BC:(i + 1) * BC]                      # [128, 3072]
        # xt[p, j*128 + b] = sh[b, j*128 + p]
        xt = np.ascontiguousarray(
            sh.reshape(BC, KCH, 128).transpose(2, 1, 0).reshape(128, KCH * BC)
        )
        in_maps.append({"xt": xt, "wc": wc, "bv": bv, "id10": id10})
    return in_maps


def kernel(data, W, b, trace=False, trace_kwargs=None):
    nc = _get_nc()
    in_maps = _shard_inputs(np.asarray(data), np.asarray(W), np.asarray(b))
    kw = {}
    if trace:
        kw = dict(trace=True, trace_cores=list(range(NCORES)),
                  stitch_traces=True)
        if trace_kwargs:
            kw["trace_kwargs"] = trace_kwargs
    res = run_bass_kernel_spmd(
        nc, in_maps, core_ids=list(range(NCORES)), **kw
    )
    regs = np.concatenate([r["reg"].reshape(-1) for r in res.results])
    mean = np.float32(regs.mean())
    out = (np.asarray(mean, np.float32), np.asarray(0, np.int32))
    if trace:
        return out, res
    return out


# revision 18
# speedup vs baseline: 1.0488x; 1.0488x over previous
"""Trainium2 Bass kernel: isometry-regularization loss (nn_IsometryReg).

Math: for a linear classifier l = xW + b (c=10 classes, n=3072 features),
the per-sample Jacobian of y = 2 r[:9] / (1 - r[9])  (r = sqrt(a*softmax(l)+eps))
w.r.t. x factors as  jac = Jl @ W^T  with Jl [9,10] the Jacobian w.r.t. logits:
    Jl[i,j] = alpha_i d_ij + gamma_i d_j9 - (alpha_i+gamma_i) s_j
    alpha_i = a u s_i / r_i,  gamma_i = a u^2 r_i s_9 / r_9,  u = 1/(1-r_9)
Hence G = jac jac^T = Jl (W^T W) Jl^T — the [B,9,3072] Jacobian is never
materialized.  ||G - f I||_F^2 = ||G||^2 - 2 f tr(G) + 9 f^2 (f >> ||G||, no
cancellation), and arccos(x) = arctan(sqrt(1-x^2)/x) for the x>0 range here.

Sharding: pure data-parallel, 128 samples per core on 8 cores; W, b replicated.
Per-core shard is sent pre-laid-out as x^T chunks (xt[p, j*128+b] =
x[b, j*128+p]) so the k-contraction lands on partitions; this is a layout
choice of the shard, the device still reads every byte of x once.
"""

import numpy as np

import concourse.bass as bass
import concourse.tile as tile
from concourse import mybir
from concourse.bass_utils import run_bass_kernel_spmd

F32 = mybir.dt.float32
AX = mybir.AxisListType
OP = mybir.AluOpType
AF = mybir.ActivationFunctionType

B, N, C = 1024, 3072, 10
M = C - 1                      # 9
NCORES = 8
BC = B // NCORES               # 128 samples per core
KCH = N // 128                 # 24 k-chunks
NUM_STAB = 1e-4
A_CONST = 1.0 - C * NUM_STAB   # 0.999
EPSILON = 0.1

_CACHE = {}

# feature toggles for walrus-codegen bisect
USE_PBCAST = True
USE_TTR = False
NDMA = 8


def _build():
    nc = bass.Bass()

    xt = nc.dram_tensor("xt", [128, N], F32, kind="ExternalInput")
    # packed consts: [:, :240]=wc, [:10, 240]=b, [:10, 241:251]=eye(10)
    wc = nc.dram_tensor("wc", [128, KCH * C + 11], F32, kind="ExternalInput")
    out = nc.dram_tensor("reg", [BC, 1], F32, kind="ExternalOutput")

    with tile.TileContext(nc) as tc:
        with (
            tc.tile_pool(name="const", bufs=1) as const,
            tc.tile_pool(name="xb", bufs=1) as xb,
            tc.tile_pool(name="work", bufs=1) as work,
            tc.tile_pool(name="psum", bufs=1, space="PSUM") as psum,
        ):
            # ---- loads ----
            wc_sb = const.tile([128, KCH * C + 11], F32)
            nc.sync.dma_start(wc_sb[:], wc[:])
            b_sb = wc_sb[0:C, KCH * C:KCH * C + 1]
            id_sb = wc_sb[0:C, KCH * C + 1:KCH * C + 11]

            xt_sb = xb.tile([128, N], F32)
            cw = N // NDMA
            for d in range(NDMA):
                nc.sync.dma_start(
                    xt_sb[:, d * cw:(d + 1) * cw], xt[:, d * cw:(d + 1) * cw]
                )

            # ---- K = W^T W  [10,10], then broadcast to [128, 100] ----
            kpsum = psum.tile([C, C], F32)
            for j in range(KCH):
                nc.tensor.matmul(
                    kpsum[:],
                    wc_sb[:, j * C:(j + 1) * C],
                    wc_sb[:, j * C:(j + 1) * C],
                    start=(j == 0),
                    stop=(j == KCH - 1),
                )
            k10_sb = const.tile([C, C], F32)
            nc.scalar.copy(k10_sb[:], kpsum[:])
            k1_sb = const.tile([1, C * C], F32)
            nc.sync.dma_start(k1_sb[:], k10_sb[:])
            kbc = const.tile([128, C * C], F32)
            # PE broadcast: ones[1,128]^T @ k1[1,100].  The warmup matmul
            # absorbs the DVE-memset dependency so the broadcast matmul
            # carries a single sync wait (f32 self-loading LDW struct has
            # one wait slot in walrus codegen).
            ones1 = const.tile([1, 128], F32)
            nc.vector.memset(ones1[:], 1.0)
            warm_ps = psum.tile([128, 1], F32)
            nc.tensor.matmul(warm_ps[:], ones1[:], ones1[:, 0:1],
                             start=True, stop=True)
            kbc_ps = psum.tile([128, C * C], F32)
            nc.tensor.matmul(kbc_ps[:], ones1[:], k1_sb[:],
                             start=True, stop=True)
            nc.scalar.copy(kbc[:], kbc_ps[:])

            # ---- logitsT = W^T x^T [10, 128] ----
            lpsum = psum.tile([C, 128], F32)
            for j in range(KCH):
                nc.tensor.matmul(
                    lpsum[:],
                    wc_sb[:, j * C:(j + 1) * C],
                    xt_sb[:, j * 128:(j + 1) * 128],
                    start=(j == 0),
                    stop=(j == KCH - 1),
                )
            lt_sb = work.tile([C, 128], F32)
            # bias add fused with PSUM->SBUF copy
            nc.vector.tensor_scalar_add(lt_sb[:], lpsum[:], b_sb)

            # ---- transpose -> logits [128, 10] ----
            l_psum = psum.tile([128, C], F32)
            nc.tensor.transpose(l_psum[:], lt_sb[:], id_sb)

            # ---- softmax (batch on partitions) ----
            negmax = work.tile([BC, 1], F32)
            nc.vector.tensor_reduce(
                negmax[:], l_psum[:], axis=AX.X, op=OP.max, negate=True
            )
            E = work.tile([BC, C], F32)
            SE = work.tile([BC, 1], F32)
            nc.scalar.activation(
                E[:], l_psum[:], AF.Exp, bias=negmax[:], scale=1.0, accum_out=SE[:]
            )
            SEr = work.tile([BC, 1], F32)
            nc.vector.reciprocal(SEr[:], SE[:])
            S = work.tile([BC, C], F32)
            nc.scalar.mul(S[:], E[:], SEr[:])

            # r = sqrt(a*s + eps), with accumulated row-sum for delta
            eps_sb = const.tile([BC, 1], F32)
            nc.vector.memset(eps_sb[:], NUM_STAB)
            R = work.tile([BC, C], F32)
            SR = work.tile([BC, 1], F32)
            nc.scalar.activation(
                R[:], S[:], AF.Sqrt, bias=eps_sb[:], scale=A_CONST, accum_out=SR[:]
            )
            Rinv = work.tile([BC, C], F32)
            nc.vector.reciprocal(Rinv[:], R[:])

            # u = 1/(1 - r9), u^2
            OMR = work.tile([BC, 1], F32)
            nc.vector.tensor_scalar(
                OMR[:], R[:, M:C], -1.0, 1.0, op0=OP.mult, op1=OP.add
            )
            U = work.tile([BC, 1], F32)
            nc.vector.reciprocal(U[:], OMR[:])
            U2 = work.tile([BC, 1], F32)
            nc.vector.tensor_mul(U2[:], U[:], U[:])

            # alpha, gamma, -(alpha+gamma)
            SRi = work.tile([BC, M], F32)
            nc.vector.tensor_mul(SRi[:], S[:, :M], Rinv[:, :M])
            ALPHA = work.tile([BC, M], F32)
            nc.vector.tensor_scalar(
                ALPHA[:], SRi[:], U[:], A_CONST, op0=OP.mult, op1=OP.mult
            )
            SR9 = work.tile([BC, 1], F32)
            nc.vector.tensor_mul(SR9[:], S[:, M:C], Rinv[:, M:C])
            G0 = work.tile([BC, 1], F32)
            nc.vector.tensor_scalar(
                G0[:], SR9[:], U2[:], A_CONST, op0=OP.mult, op1=OP.mult
            )
            GAMMA = work.tile([BC, M], F32)
            nc.vector.tensor_scalar_mul(GAMMA[:], R[:, :M], G0[:])
            TAUN = work.tile([BC, M], F32)
            nc.vector.scalar_tensor_tensor(
                TAUN[:], ALPHA[:], -1.0, GAMMA[:], op0=OP.mult, op1=OP.subtract
            )

            # ---- Jl [128, 90]:  -(tau) x s  + diag(alpha) + gamma e9 ----
            JL = work.tile([BC, M * C], F32)
            nc.vector.tensor_mul(
                JL[:].rearrange("p (i j) -> p i j", i=M),
                TAUN[:, :, None].broadcast_to([BC, M, C]),
                S[:, None, :].broadcast_to([BC, M, C]),
            )
            nc.vector.tensor_add(JL[:, 0:M * C:C + 1], JL[:, 0:M * C:C + 1], ALPHA[:])
            nc.vector.tensor_add(
                JL[:, M:M * C:C], JL[:, M:M * C:C], GAMMA[:]
            )

            # ---- TT = Jl K  (per sample): [128, 90] ----
            TTm = work.tile([BC, M * C * C], F32)
            nc.vector.tensor_mul(
                TTm[:].rearrange("p (i k j) -> p i k j", i=M, k=C),
                JL[:].rearrange("p (i j) -> p i j", i=M)[:, :, None, :]
                .broadcast_to([BC, M, C, C]),
                kbc[:].rearrange("p (k j) -> p k j", k=C)[:, None, :, :]
                .broadcast_to([BC, M, C, C]),
            )
            TT = work.tile([BC, M * C], F32)
            nc.vector.tensor_reduce(
                TT[:], TTm[:].rearrange("p (g j) -> p g j", j=C),
                axis=AX.X, op=OP.add,
            )

            # ---- G = TT Jl^T (per sample): [128, 81] ----
            Gm = work.tile([BC, M * M * C], F32)
            nc.vector.tensor_mul(
                Gm[:].rearrange("p (i l k) -> p i l k", i=M, l=M),
                TT[:].rearrange("p (i k) -> p i k", i=M)[:, :, None, :]
                .broadcast_to([BC, M, M, C]),
                JL[:].rearrange("p (l k) -> p l k", l=M)[:, None, :, :]
                .broadcast_to([BC, M, M, C]),
            )
            G = work.tile([BC, M * M], F32)
            nc.vector.tensor_reduce(
                G[:], Gm[:].rearrange("p (g k) -> p g k", k=C), axis=AX.X, op=OP.add
            )

            # ---- ||G||^2 and tr(G) ----
            scrap = work.tile([BC, M * M], F32)
            SSQ = work.tile([BC, 1], F32)
            if USE_TTR:
                nc.vector.tensor_tensor_reduce(
                    out=scrap[:], in0=G[:], in1=G[:], scale=1.0, scalar=0.0,
                    op0=OP.mult, op1=OP.add, accum_out=SSQ[:],
                )
            else:
                nc.vector.tensor_mul(scrap[:], G[:], G[:])
                nc.vector.tensor_reduce(SSQ[:], scrap[:], axis=AX.X, op=OP.add)
            TRG = work.tile([BC, 1], F32)
            nc.vector.tensor_reduce(
                TRG[:], G[:, 0:M * M:M + 1], axis=AX.X, op=OP.add
            )

            # ---- delta = 2 arccos(SR/sqrt(10)) via arctan ----
            X2 = work.tile([BC, 1], F32)
            nc.scalar.activation(X2[:], SR[:], AF.Square, scale=1.0 / np.sqrt(C))
            OMX2 = work.tile([BC, 1], F32)
            nc.vector.tensor_scalar(
                OMX2[:], X2[:], -1.0, 1.0, op0=OP.mult, op1=OP.add
            )
            SQX = work.tile([BC, 1], F32)
            nc.scalar.activation(SQX[:], OMX2[:], AF.Sqrt)
            XV = work.tile([BC, 1], F32)
            nc.vector.tensor_scalar_mul(XV[:], SR[:], float(1.0 / np.sqrt(C)))
            XR = work.tile([BC, 1], F32)
            nc.vector.reciprocal(XR[:], XV[:])
            QT = work.tile([BC, 1], F32)
            nc.vector.tensor_mul(QT[:], SQX[:], XR[:])
            AC = work.tile([BC, 1], F32)
            nc.scalar.activation(AC[:], QT[:], AF.Arctan)

            # ---- f = 100 * AC^2 * u^2 ; res = SSQ - 2 f trG + 9 f^2 ----
            FA = work.tile([BC, 1], F32)
            nc.vector.tensor_mul(FA[:], AC[:], AC[:])
            F = work.tile([BC, 1], F32)
            nc.vector.tensor_scalar(
                F[:], FA[:], U2[:], 100.0, op0=OP.mult, op1=OP.mult
            )
            FT = work.tile([BC, 1], F32)
            nc.vector.tensor_mul(FT[:], F[:], TRG[:])
            R1 = work.tile([BC, 1], F32)
            nc.vector.scalar_tensor_tensor(
                R1[:], FT[:], -2.0, SSQ[:], op0=OP.mult, op1=OP.add
            )
            FF = work.tile([BC, 1], F32)
            nc.vector.tensor_mul(FF[:], F[:], F[:])
            RES = work.tile([BC, 1], F32)
            nc.vector.scalar_tensor_tensor(
                RES[:], FF[:], 9.0, R1[:], op0=OP.mult, op1=OP.add
            )
            REG = work.tile([BC, 1], F32)
            nc.scalar.activation(
                REG[:], RES[:], AF.Sqrt, scale=1.0 / (float(N) * float(N))
            )
            nc.sync.dma_start(out[:], REG[:])

    return nc


def _split_waits(nc):
    """Walrus codegen on this toolchain encodes at most one sync-wait per
    instruction; hoist extra waits onto same-engine NoOps inserted before."""
    for blk in nc.main_func.blocks:
        newlist = []
        changed = False
        for ins in blk.instructions:
            si = getattr(ins, "sync_info", None)
            ow = getattr(si, "on_wait", None) if si is not None else None
            if ow and len(ow) > 1:
                for idx, w in enumerate(ow[:-1]):
                    nop = mybir.InstNoOp(name=f"{ins.name}-sw{idx}", ins=[], outs=[])
                    nop.engine = ins.engine
                    nop.sync_info = mybir.SyncInfo(on_wait=[w], on_update=[])
                    newlist.append(nop)
                si.on_wait = [ow[-1]]
                changed = True
            newlist.append(ins)
        if changed:
            blk.instructions = newlist
    return nc


def _get_nc():
    if "nc" not in _CACHE:
        _CACHE["nc"] = _split_waits(_build())
    return _CACHE["nc"]


def _shard_inputs(data, W, b):
    """Host-side layout: per-core transposed x chunks + chunked W."""
    x = np.ascontiguousarray(np.asarray(data, np.float32).reshape(B, N))
    W = np.asarray(W, np.float32)
    b = np.asarray(b, np.float32)

    # packed consts: wc[p, j*10+c] = W[j*128+p, c]; col 240 = b; 241:251 = I
    wc = np.zeros((128, KCH * C + 11), np.float32)
    wc[:, :KCH * C] = (
        W.reshape(KCH, 128, C).transpose(1, 0, 2).reshape(128, KCH * C)
    )
    wc[:C, KCH * C] = b
    wc[:C, KCH * C + 1:] = np.eye(C, dtype=np.float32)

    in_maps = []
    for i in range(NCORES):
        sh = x[i * BC:(i + 1) * BC]                      # [128, 3072]
        # xt[p, j*128 + b] = sh[b, j*128 + p]
        xt = np.ascontiguousarray(
            sh.reshape(BC, KCH, 128).transpose(2, 1, 0).reshape(128, KCH * BC)
        )
        in_maps.append({"xt": xt, "wc": wc})
    return in_maps


def kernel(data, W, b, trace=False, trace_kwargs=None):
    nc = _get_nc()
    in_maps = _shard_inputs(np.asarray(data), np.asarray(W), np.asarray(b))
    kw = {}
    if trace:
        kw = dict(trace=True, trace_cores=list(range(NCORES)),
                  stitch_traces=True)
        if trace_kwargs:
            kw["trace_kwargs"] = trace_kwargs
    res = run_bass_kernel_spmd(
        nc, in_maps, core_ids=list(range(NCORES)), **kw
    )
    regs = np.concatenate([r["reg"].reshape(-1) for r in res.results])
    mean = np.float32(regs.mean())
    out = (np.asarray(mean, np.float32), np.asarray(0, np.int32))
    if trace:
        return out, res
    return out
